# revision 1
# baseline (speedup 1.0000x reference)
"""GRU cell (B=4096, H=2048) on 8 TRN2 NeuronCores.

Sharding: data-parallel over the batch dim — each core computes 512 rows.
Weights are replicated; no collectives.

Per-core compute runs in "transposed" space (hidden on partitions, batch on
the free dim): for hidden block nb (128 units) the three gate pre-activations
are built by PSUM accumulation

    psum = sum_k W[k*128:(k+1)*128, nb*128:(nb+1)*128]^T @ actT[k]

with float32r (FP22 reduced-precision fp32) matmuls at free-dim 512, which run
at full PE rate. r/z gates accumulate the ih and hh contributions into a
single PSUM bank; the n gate keeps gi2/gh2 separate (needed for gi2 + r*gh2).
Biases become per-partition scalars in this layout, so ScalarE fuses them into
the sigmoid/tanh activation. The host pre-transposes the activation shards and
packs the weights so every weight DMA is one contiguous 1 MiB slab.
"""

from contextlib import ExitStack

import ml_dtypes
import numpy as np

import concourse.bass as bass
import concourse.tile as tile
from concourse import bacc, mybir
from concourse.bass_utils import run_bass_kernel_spmd

H = 2048
B = 4096
N_CORES = 8
BL = B // N_CORES  # 512 batch rows per core
P = 128
NKB = H // P  # 16 contraction chunks
NNB = H // P  # 16 hidden (output) blocks
F32 = mybir.dt.float32
F32R = mybir.dt.float32r
BF16 = mybir.dt.bfloat16

# Weight matrix order in the packed tensor: (gate, ih/hh)
# 0: W_ih[0] (r)   1: W_hh[0] (r)
# 2: W_ih[1] (z)   3: W_hh[1] (z)
# 4: W_ih[2] (n)   5: W_hh[2] (n)


def _build_program() -> bacc.Bacc:
    nc = bacc.Bacc(
        "TRN2", target_bir_lowering=False, debug=False, num_devices=N_CORES
    )

    # float32r (reduced-precision fp32) end-to-end on the matmul operand
    # path: the BIR verifier requires matmul inputs to be produced as f32r.
    # Same bits as f32; numpy binding is float32.
    xt = nc.dram_tensor("xt", [P, NKB * BL], F32R, kind="ExternalInput").ap()
    hxt = nc.dram_tensor("hxt", [P, NKB * BL], F32R, kind="ExternalInput").ap()
    w = nc.dram_tensor("w", [6, NNB, P, H], F32R, kind="ExternalInput").ap()
    b = nc.dram_tensor("b", [P, 4 * NNB], F32, kind="ExternalInput").ap()
    out = nc.dram_tensor("out", [H, BL], F32, kind="ExternalOutput").ap()

    with tile.TileContext(nc) as tc, ExitStack() as ctx:
        const = ctx.enter_context(tc.tile_pool(name="const", bufs=1))
        acts = ctx.enter_context(tc.tile_pool(name="acts", bufs=1))
        wpool = ctx.enter_context(tc.tile_pool(name="wpool", bufs=10))
        gates = ctx.enter_context(tc.tile_pool(name="gates", bufs=2))
        opool = ctx.enter_context(tc.tile_pool(name="opool", bufs=3))
        ps_r = ctx.enter_context(tc.tile_pool(name="ps_r", bufs=2, space="PSUM"))
        ps_z = ctx.enter_context(tc.tile_pool(name="ps_z", bufs=2, space="PSUM"))
        ps_gi = ctx.enter_context(tc.tile_pool(name="ps_gi", bufs=2, space="PSUM"))
        ps_gh = ctx.enter_context(tc.tile_pool(name="ps_gh", bufs=2, space="PSUM"))

        # Startup: one serial need-ordered stream on the sync ring — total
        # startup bytes are HBM-bound, so parallel rings only reshuffle the
        # stalls; serial delivery in consumption order minimizes them.
        btile = const.tile([P, 4 * NNB], F32)
        nc.scalar.dma_start(btile[:], b[:])
        xt_sb = acts.tile([P, NKB * BL], F32R)
        hxt_sb = acts.tile([P, NKB * BL], F32R)
        nb0_slabs = [None] * 6
        CH = 8 * BL  # 2 MiB activation chunks (8 k-blocks each)
        for c in range(2):
            nc.sync.dma_start(
                xt_sb[:, c * CH : (c + 1) * CH], xt[:, c * CH : (c + 1) * CH]
            )
        for m in (0, 2, 4):
            s = wpool.tile([P, H], F32R, tag="wslab", name=f"w{m}_0")
            nc.sync.dma_start(s[:], w[m, 0])
            nb0_slabs[m] = s
        # hxt chunk 0 + w1 land before hxt chunk 1: the first 8 hh matmuls
        # only read hxt[:, :CH], so the hh half starts one slab-time earlier
        # (Tile's RAW deps are AP-range-granular).
        nc.sync.dma_start(hxt_sb[:, 0:CH], hxt[:, 0:CH])
        s = wpool.tile([P, H], F32R, tag="wslab", name="w1_0")
        nc.sync.dma_start(s[:], w[1, 0])
        nb0_slabs[1] = s
        nc.sync.dma_start(hxt_sb[:, CH : 2 * CH], hxt[:, CH : 2 * CH])
        for m in (3, 5):
            s = wpool.tile([P, H], F32R, tag="wslab", name=f"w{m}_0")
            nc.sync.dma_start(s[:], w[m, 0])
            nb0_slabs[m] = s

        # PE warm-up: throwaway matmuls on a memset tile release the HAM
        # clock gate and keep the PE busy while the startup DMAs land, so
        # the first real matmuls run at 2.4 GHz.
        warm = const.tile([P, BL], BF16)
        nc.gpsimd.memset(warm[:], 0.0)
        p_warm = ps_gh.tile([P, BL], F32, tag="p_gh", name="p_warm")

        def warm_mms(n):
            for _ in range(n):
                nc.tensor.matmul(
                    p_warm[:], lhsT=warm[:, :P], rhs=warm[:],
                    start=True, stop=True,
                )

        warm_mms(40)

        def mm_half(psum, slab, act_sb, start, stop):
            """One 16-matmul K sweep accumulated into psum."""
            for k in range(NKB):
                nc.tensor.matmul(
                    psum[:],
                    lhsT=slab[:, k * P : (k + 1) * P],
                    rhs=act_sb[:, k * BL : (k + 1) * BL],
                    start=(start and k == 0),
                    stop=(stop and k == NKB - 1),
                )

        for nb in range(NNB):
            sl = [None] * 6
            order = (0, 2, 4, 1, 3, 5) if nb == 0 else (4, 5, 0, 1, 2, 3)
            for m in order:
                if nb == 0:
                    sl[m] = nb0_slabs[m]
                    continue
                s = wpool.tile([P, H], F32R, tag="wslab", name=f"w{m}_{nb}")
                nc.sync.dma_start(s[:], w[m, nb])
                sl[m] = s

            p_r = ps_r.tile([P, BL], F32)
            p_z = ps_z.tile([P, BL], F32)
            p_gi = ps_gi.tile([P, BL], F32)
            p_gh = ps_gh.tile([P, BL], F32)
            if nb == 0:
                # xt-only halves first so the PE can start before hxt lands
                mm_half(p_r, sl[0], xt_sb, start=True, stop=False)
                mm_half(p_z, sl[2], xt_sb, start=True, stop=False)
                mm_half(p_gi, sl[4], xt_sb, start=True, stop=True)
                warm_mms(18)
                mm_half(p_r, sl[1], hxt_sb, start=False, stop=True)
                mm_half(p_z, sl[3], hxt_sb, start=False, stop=True)
                mm_half(p_gh, sl[5], hxt_sb, start=True, stop=True)
            else:
                # n-gate first: its tanh chain overlaps the r/z matmuls,
                # leaving only sigmoid -> mul -> add after the last matmul.
                mm_half(p_gi, sl[4], xt_sb, start=True, stop=True)
                mm_half(p_gh, sl[5], hxt_sb, start=True, stop=True)
                mm_half(p_r, sl[0], xt_sb, start=True, stop=False)
                mm_half(p_r, sl[1], hxt_sb, start=False, stop=True)
                mm_half(p_z, sl[2], xt_sb, start=True, stop=False)
                mm_half(p_z, sl[3], hxt_sb, start=False, stop=True)

            def bias_ap(g):
                return btile[:, g * NNB + nb : g * NNB + nb + 1]

            # r = sigmoid(gi0 + gh0 + b_ih0 + b_hh0)
            r_sb = gates.tile([P, BL], F32, tag="r")
            nc.scalar.activation(
                r_sb[:], p_r[:], mybir.ActivationFunctionType.Sigmoid,
                bias=bias_ap(0),
            )
            # z = sigmoid(gi1 + gh1 + b_ih1 + b_hh1); halved for the last
            # block so the z -> e -> o -> DMA chain pipelines across engines
            z_sb = gates.tile([P, BL], F32, tag="z")
            z_halves = 2 if nb == NNB - 1 else 1
            ZH = BL // z_halves
            for zh in range(z_halves):
                nc.scalar.activation(
                    z_sb[:, zh * ZH : (zh + 1) * ZH],
                    p_z[:, zh * ZH : (zh + 1) * ZH],
                    mybir.ActivationFunctionType.Sigmoid,
                    bias=bias_ap(1),
                )
            # t = (gh2 + b_hh2) * r
            t_sb = gates.tile([P, BL], F32, tag="t")
            nc.vector.scalar_tensor_tensor(
                t_sb[:], p_gh[:], bias_ap(3), r_sb[:],
                op0=mybir.AluOpType.add, op1=mybir.AluOpType.mult,
            )
            # n = tanh(gi2 + b_ih2 + t)
            x_sb = gates.tile([P, BL], F32, tag="x")
            nc.vector.tensor_add(x_sb[:], t_sb[:], p_gi[:])
            n_sb = gates.tile([P, BL], F32, tag="n")
            nc.scalar.activation(
                n_sb[:], x_sb[:], mybir.ActivationFunctionType.Tanh,
                bias=bias_ap(2),
            )
            # out = n + z * (hx - n)
            d_sb = gates.tile([P, BL], F32, tag="d")
            nc.vector.tensor_sub(
                d_sb[:], hxt_sb[:, nb * BL : (nb + 1) * BL].bitcast(F32), n_sb[:]
            )
            e_sb = gates.tile([P, BL], F32, tag="e")
            o_sb = opool.tile([P, BL], F32, tag="o")
            for zh in range(z_halves):
                hs = slice(zh * ZH, (zh + 1) * ZH)
                nc.vector.tensor_mul(e_sb[:, hs], z_sb[:, hs], d_sb[:, hs])
                nc.vector.tensor_add(o_sb[:, hs], n_sb[:, hs], e_sb[:, hs])
                if nb == NNB - 1:
                    nc.sync.dma_start(out[nb * P : (nb + 1) * P, hs], o_sb[:, hs])
            if nb != NNB - 1:
                nc.gpsimd.dma_start(out[nb * P : (nb + 1) * P, :], o_sb[:])

    nc.compile()
    return nc


def _pack_inputs(input, hx, weight_ih, weight_hh, bias_ih, bias_hh):
    """Host-side shard + layout packing. Returns per-core input maps."""
    input = np.ascontiguousarray(np.asarray(input, dtype=np.float32))
    hx = np.ascontiguousarray(np.asarray(hx, dtype=np.float32))
    weight_ih = np.asarray(weight_ih, dtype=np.float32)
    weight_hh = np.asarray(weight_hh, dtype=np.float32)
    bias_ih = np.asarray(bias_ih, dtype=np.float32)
    bias_hh = np.asarray(bias_hh, dtype=np.float32)

    # wpack[m, nb, kp, k*128+n] = W_m[k*128+kp, nb*128+n]
    ws = [weight_ih[0], weight_hh[0], weight_ih[1], weight_hh[1],
          weight_ih[2], weight_hh[2]]
    wpack = np.ascontiguousarray(
        np.stack(
            [wm.reshape(NKB, P, NNB, P).transpose(2, 1, 0, 3) for wm in ws]
        ).reshape(6, NNB, P, H)
    )

    # bpack[p, g*16+nb] = bias_g[nb*128+p];  g order: r_sum, z_sum, ih2, hh2
    bias_all = np.stack(
        [bias_ih[0] + bias_hh[0], bias_ih[1] + bias_hh[1], bias_ih[2], bias_hh[2]]
    )  # [4, H]
    bpack = np.ascontiguousarray(
        bias_all.reshape(4, NNB, P).transpose(2, 0, 1).reshape(P, 4 * NNB)
    )

    def t_pack(a, dt=np.float32):
        # [BL, H] -> [P, NKB*BL] with [kp, k*BL+m] = a[m, k*128+kp]
        return np.ascontiguousarray(
            a.T.reshape(NKB, P, BL).transpose(1, 0, 2).reshape(P, NKB * BL)
            .astype(dt)
        )

    in_maps = []
    for c in range(N_CORES):
        sl = slice(c * BL, (c + 1) * BL)
        in_maps.append(
            {
                "xt": t_pack(input[sl]),
                "hxt": t_pack(hx[sl]),
                "w": wpack,
                "b": bpack,
            }
        )
    return in_maps


_PROGRAM_CACHE = []


def kernel(input, hx, weight_ih, weight_hh, bias_ih, bias_hh, _trace=False):
    if not _PROGRAM_CACHE:
        _PROGRAM_CACHE.append(_build_program())
    nc = _PROGRAM_CACHE[0]
    in_maps = _pack_inputs(input, hx, weight_ih, weight_hh, bias_ih, bias_hh)
    res = run_bass_kernel_spmd(nc, in_maps, list(range(N_CORES)), trace=_trace)
    out = np.empty((B, H), dtype=np.float32)
    for c in range(N_CORES):
        out[c * BL : (c + 1) * BL] = res.results[c]["out"].T
    if _trace:
        kernel.last_exec_time_ns = res.exec_time_ns
    return out



# revision 2
# speedup vs baseline: 1.5167x; 1.5167x over previous
"""GRU cell (B=4096, H=2048) on 8 TRN2 NeuronCores.

Sharding: data-parallel over the batch dim — each core computes 512 rows.
Weights are replicated; no collectives.

Per-core compute runs in "transposed" space (hidden on partitions, batch on
the free dim). Precision strategy (gate on rel-err < 2e-2; measured 1.3e-2
in numpy simulation):
  - r/z gates: fp8-e4m3 DoubleRow matmuls (2 contraction rows per PE cell,
    2x MAC rate). Inputs scaled by SX=32 (acts) / SW=8192 (weights) to sit
    in e4m3's normal range; the 1/(SX*SW) descale folds into the sigmoid's
    scale operand on ScalarE. Sigmoid squashes the quantization error.
  - n gate: bf16 matmuls (full rate) — tanh passes error 1:1, fp8 would
    blow the error budget.
This also cuts weight DMA 3x (2 MiB/block vs 6 MiB), taking the DMA stream
well below the PE roofline (baseline ran both at ~90% busy).

For hidden block nb (128 units): r/z accumulate ih+hh into one PSUM bank
via 16 DoubleRow matmuls (K=256 each); gi2/gh2 use 16 bf16 matmuls each.
Biases are per-partition scalars fused into ScalarE activations.
"""

from contextlib import ExitStack

import ml_dtypes
import numpy as np

import concourse.bass as bass
import concourse.tile as tile
from concourse import bacc, mybir
from concourse.bass_utils import run_bass_kernel_spmd

H = 2048
B = 4096
N_CORES = 8
BL = B // N_CORES  # 512 batch rows per core
P = 128
NKB = H // P  # 16 contraction chunks of 128
ND = NKB // 2  # 8 DoubleRow chunks of 256
NNB = H // P  # 16 hidden (output) blocks
F32 = mybir.dt.float32
BF16 = mybir.dt.bfloat16
F8 = mybir.dt.float8e4

SX = 32.0  # activation quant scale
SW = 8192.0  # weight quant scale
SINV = 1.0 / (SX * SW)
F8MAX = 240.0  # TRN FP8_EXP4 max normal

# fp8 weight matrix order: 0: W_ih[0] (r)  1: W_hh[0] (r)
#                          2: W_ih[1] (z)  3: W_hh[1] (z)
# bf16 weight order:       0: W_ih[2] (n)  1: W_hh[2] (n)


def _build_program() -> bacc.Bacc:
    nc = bacc.Bacc(
        "TRN2", target_bir_lowering=False, debug=False, num_devices=N_CORES
    )

    xq8 = nc.dram_tensor("xq8", [P, NKB, BL], F8, kind="ExternalInput").ap()
    hq8 = nc.dram_tensor("hq8", [P, NKB, BL], F8, kind="ExternalInput").ap()
    xb = nc.dram_tensor("xb", [P, NKB, BL], BF16, kind="ExternalInput").ap()
    hb = nc.dram_tensor("hb", [P, NKB, BL], BF16, kind="ExternalInput").ap()
    w8 = nc.dram_tensor("w8", [4, NNB, P, NKB, P], F8, kind="ExternalInput").ap()
    wb = nc.dram_tensor("wb", [2, NNB, P, NKB, P], BF16, kind="ExternalInput").ap()
    b = nc.dram_tensor("b", [P, 4 * NNB], F32, kind="ExternalInput").ap()
    out = nc.dram_tensor("out", [H, BL], F32, kind="ExternalOutput").ap()

    with tile.TileContext(nc) as tc, ExitStack() as ctx:
        const = ctx.enter_context(tc.tile_pool(name="const", bufs=1))
        acts = ctx.enter_context(tc.tile_pool(name="acts", bufs=1))
        w8pool = ctx.enter_context(tc.tile_pool(name="w8pool", bufs=10))
        wbpool = ctx.enter_context(tc.tile_pool(name="wbpool", bufs=6))
        gates = ctx.enter_context(tc.tile_pool(name="gates", bufs=2))
        opool = ctx.enter_context(tc.tile_pool(name="opool", bufs=3))
        ps_r = ctx.enter_context(tc.tile_pool(name="ps_r", bufs=2, space="PSUM"))
        ps_z = ctx.enter_context(tc.tile_pool(name="ps_z", bufs=2, space="PSUM"))
        ps_gi = ctx.enter_context(tc.tile_pool(name="ps_gi", bufs=2, space="PSUM"))
        ps_gh = ctx.enter_context(tc.tile_pool(name="ps_gh", bufs=2, space="PSUM"))

        # Startup: serial need-ordered stream on the sync ring. fp8 r/z
        # operands first so DoubleRow matmuls start after ~1.5 MiB; bf16
        # n-gate operands stream behind them.
        btile = const.tile([P, 4 * NNB], F32)
        nc.scalar.dma_start(btile[:], b[:])
        xq8_sb = acts.tile([P, NKB, BL], F8)
        hq8_sb = acts.tile([P, NKB, BL], F8)
        xb_sb = acts.tile([P, NKB, BL], BF16)
        hb_sb = acts.tile([P, NKB, BL], BF16)
        nb0_s8 = [None] * 4
        nb0_sb = [None] * 2

        nc.sync.dma_start(xq8_sb[:], xq8[:])
        for m in (0, 2):
            s = w8pool.tile([P, NKB, P], F8, tag="wslab8", name=f"w8_{m}_0")
            nc.sync.dma_start(s[:], w8[m, 0])
            nb0_s8[m] = s
        nc.sync.dma_start(hq8_sb[:], hq8[:])
        for m in (1, 3):
            s = w8pool.tile([P, NKB, P], F8, tag="wslab8", name=f"w8_{m}_0")
            nc.sync.dma_start(s[:], w8[m, 0])
            nb0_s8[m] = s
        CH = 8  # k-chunks per DMA piece
        for c in range(2):
            nc.sync.dma_start(
                xb_sb[:, c * CH : (c + 1) * CH, :], xb[:, c * CH : (c + 1) * CH, :]
            )
        s = wbpool.tile([P, NKB, P], BF16, tag="wslabb", name="wb_0_0")
        nc.sync.dma_start(s[:], wb[0, 0])
        nb0_sb[0] = s
        for c in range(2):
            nc.sync.dma_start(
                hb_sb[:, c * CH : (c + 1) * CH, :], hb[:, c * CH : (c + 1) * CH, :]
            )
        s = wbpool.tile([P, NKB, P], BF16, tag="wslabb", name="wb_1_0")
        nc.sync.dma_start(s[:], wb[1, 0])
        nb0_sb[1] = s

        # PE warm-up: throwaway matmuls on a memset tile release the HAM
        # clock gate and keep the PE busy while the startup DMAs land.
        warm = const.tile([P, BL], BF16)
        nc.gpsimd.memset(warm[:], 0.0)
        p_warm = ps_gh.tile([P, BL], F32, tag="p_gh", name="p_warm")

        def warm_mms(n):
            for _ in range(n):
                nc.tensor.matmul(
                    p_warm[:], lhsT=warm[:, :P], rhs=warm[:],
                    start=True, stop=True,
                )

        warm_mms(40)

        def mm_dr(psum, slab, act_sb, start, stop):
            """fp8 DoubleRow half: 8 matmuls, K=256 each."""
            for k2 in range(ND):
                nc.tensor.matmul(
                    psum[:],
                    lhsT=slab[:, 2 * k2 : 2 * k2 + 2, :],
                    rhs=act_sb[:, 2 * k2 : 2 * k2 + 2, :],
                    start=(start and k2 == 0),
                    stop=(stop and k2 == ND - 1),
                    perf_mode=mybir.MatmulPerfMode.DoubleRow,
                )

        def mm_bf(psum, slab, act_sb, start, stop):
            """bf16 half: 16 matmuls, K=128 each."""
            for k in range(NKB):
                nc.tensor.matmul(
                    psum[:],
                    lhsT=slab[:, k : k + 1, :],
                    rhs=act_sb[:, k : k + 1, :],
                    start=(start and k == 0),
                    stop=(stop and k == NKB - 1),
                )

        for nb in range(NNB):
            sl8 = [None] * 4
            slb = [None] * 2
            if nb == 0:
                sl8 = nb0_s8
                slb = nb0_sb
            else:
                # DMA order matches consumption order below.
                for m in (0, 1):
                    s = wbpool.tile([P, NKB, P], BF16, tag="wslabb",
                                    name=f"wb_{m}_{nb}")
                    nc.sync.dma_start(s[:], wb[m, nb])
                    slb[m] = s
                for m in range(4):
                    s = w8pool.tile([P, NKB, P], F8, tag="wslab8",
                                    name=f"w8_{m}_{nb}")
                    nc.sync.dma_start(s[:], w8[m, nb])
                    sl8[m] = s

            p_r = ps_r.tile([P, BL], F32)
            p_z = ps_z.tile([P, BL], F32)
            p_gi = ps_gi.tile([P, BL], F32)
            p_gh = ps_gh.tile([P, BL], F32)
            if nb == 0:
                # fp8 operands land first: run r/z while bf16 acts stream.
                mm_dr(p_r, sl8[0], xq8_sb, True, False)
                mm_dr(p_z, sl8[2], xq8_sb, True, False)
                mm_dr(p_r, sl8[1], hq8_sb, False, True)
                mm_dr(p_z, sl8[3], hq8_sb, False, True)
                warm_mms(16)
                mm_bf(p_gi, slb[0], xb_sb, True, True)
                mm_bf(p_gh, slb[1], hb_sb, True, True)
            else:
                # n-gate first: its tanh chain overlaps the r/z matmuls,
                # leaving only sigmoid -> mul -> add after the last matmul.
                mm_bf(p_gi, slb[0], xb_sb, True, True)
                mm_bf(p_gh, slb[1], hb_sb, True, True)
                mm_dr(p_r, sl8[0], xq8_sb, True, False)
                mm_dr(p_r, sl8[1], hq8_sb, False, True)
                mm_dr(p_z, sl8[2], xq8_sb, True, False)
                mm_dr(p_z, sl8[3], hq8_sb, False, True)

            def bias_ap(g):
                return btile[:, g * NNB + nb : g * NNB + nb + 1]

            # r = sigmoid(psum/S + b_ih0 + b_hh0)
            r_sb = gates.tile([P, BL], F32, tag="r")
            nc.scalar.activation(
                r_sb[:], p_r[:], mybir.ActivationFunctionType.Sigmoid,
                bias=bias_ap(0), scale=SINV,
            )
            # z = sigmoid(psum/S + b_ih1 + b_hh1); halved for the last
            # block so the z -> e -> o -> DMA chain pipelines across engines
            z_sb = gates.tile([P, BL], F32, tag="z")
            z_halves = 2 if nb == NNB - 1 else 1
            ZH = BL // z_halves
            for zh in range(z_halves):
                nc.scalar.activation(
                    z_sb[:, zh * ZH : (zh + 1) * ZH],
                    p_z[:, zh * ZH : (zh + 1) * ZH],
                    mybir.ActivationFunctionType.Sigmoid,
                    bias=bias_ap(1), scale=SINV,
                )
            # t = (gh2 + b_hh2) * r
            t_sb = gates.tile([P, BL], F32, tag="t")
            nc.vector.scalar_tensor_tensor(
                t_sb[:], p_gh[:], bias_ap(3), r_sb[:],
                op0=mybir.AluOpType.add, op1=mybir.AluOpType.mult,
            )
            # n = tanh(gi2 + b_ih2 + t)
            x_sb = gates.tile([P, BL], F32, tag="x")
            nc.vector.tensor_add(x_sb[:], t_sb[:], p_gi[:])
            n_sb = gates.tile([P, BL], F32, tag="n")
            nc.scalar.activation(
                n_sb[:], x_sb[:], mybir.ActivationFunctionType.Tanh,
                bias=bias_ap(2),
            )
            # out = n + z * (hx - n)
            d_sb = gates.tile([P, BL], F32, tag="d")
            nc.vector.tensor_sub(d_sb[:], hb_sb[:, nb : nb + 1, :], n_sb[:])
            e_sb = gates.tile([P, BL], F32, tag="e")
            o_sb = opool.tile([P, BL], F32, tag="o")
            for zh in range(z_halves):
                hs = slice(zh * ZH, (zh + 1) * ZH)
                nc.vector.tensor_mul(e_sb[:, hs], z_sb[:, hs], d_sb[:, hs])
                nc.vector.tensor_add(o_sb[:, hs], n_sb[:, hs], e_sb[:, hs])
                if nb == NNB - 1:
                    nc.sync.dma_start(out[nb * P : (nb + 1) * P, hs], o_sb[:, hs])
            if nb != NNB - 1:
                nc.gpsimd.dma_start(out[nb * P : (nb + 1) * P, :], o_sb[:])

    nc.compile()
    return nc


def _pack_inputs(input, hx, weight_ih, weight_hh, bias_ih, bias_hh):
    """Host-side shard + quantize + layout packing. Per-core input maps."""
    input = np.ascontiguousarray(np.asarray(input, dtype=np.float32))
    hx = np.ascontiguousarray(np.asarray(hx, dtype=np.float32))
    weight_ih = np.asarray(weight_ih, dtype=np.float32)
    weight_hh = np.asarray(weight_hh, dtype=np.float32)
    bias_ih = np.asarray(bias_ih, dtype=np.float32)
    bias_hh = np.asarray(bias_hh, dtype=np.float32)

    E4 = ml_dtypes.float8_e4m3

    # fp8 r/z weights: [m, nb, kp, k, n] = Wm[k*128+kp, nb*128+n] * SW
    ws8 = [weight_ih[0], weight_hh[0], weight_ih[1], weight_hh[1]]
    w8pack = np.ascontiguousarray(
        np.stack(
            [
                np.clip(wm * SW, -F8MAX, F8MAX)
                .reshape(NKB, P, NNB, P)
                .transpose(2, 1, 0, 3)
                for wm in ws8
            ]
        ).astype(E4)
    )
    # bf16 n-gate weights
    wbpack = np.ascontiguousarray(
        np.stack(
            [
                wm.reshape(NKB, P, NNB, P).transpose(2, 1, 0, 3)
                for wm in (weight_ih[2], weight_hh[2])
            ]
        ).astype(ml_dtypes.bfloat16)
    )

    # bpack[p, g*16+nb] = bias_g[nb*128+p];  g order: r_sum, z_sum, ih2, hh2
    bias_all = np.stack(
        [bias_ih[0] + bias_hh[0], bias_ih[1] + bias_hh[1], bias_ih[2], bias_hh[2]]
    )  # [4, H]
    bpack = np.ascontiguousarray(
        bias_all.reshape(4, NNB, P).transpose(2, 0, 1).reshape(P, 4 * NNB)
    )

    def t_pack(a, dt, scale=None):
        # [BL, H] -> [P, NKB, BL] with [p, k, m] = a[m, k*128+p]
        t = a.T.reshape(NKB, P, BL).transpose(1, 0, 2)
        if scale is not None:
            t = np.clip(t * scale, -F8MAX, F8MAX)
        return np.ascontiguousarray(t.astype(dt))

    in_maps = []
    for c in range(N_CORES):
        sl = slice(c * BL, (c + 1) * BL)
        in_maps.append(
            {
                "xq8": t_pack(input[sl], E4, SX),
                "hq8": t_pack(hx[sl], E4, SX),
                "xb": t_pack(input[sl], ml_dtypes.bfloat16),
                "hb": t_pack(hx[sl], ml_dtypes.bfloat16),
                "w8": w8pack,
                "wb": wbpack,
                "b": bpack,
            }
        )
    return in_maps


_PROGRAM_CACHE = []


def kernel(input, hx, weight_ih, weight_hh, bias_ih, bias_hh, _trace=False):
    if not _PROGRAM_CACHE:
        _PROGRAM_CACHE.append(_build_program())
    nc = _PROGRAM_CACHE[0]
    in_maps = _pack_inputs(input, hx, weight_ih, weight_hh, bias_ih, bias_hh)
    res = run_bass_kernel_spmd(nc, in_maps, list(range(N_CORES)), trace=_trace)
    out = np.empty((B, H), dtype=np.float32)
    for c in range(N_CORES):
        out[c * BL : (c + 1) * BL] = res.results[c]["out"].T
    if _trace:
        kernel.last_exec_time_ns = res.exec_time_ns
    return out


# revision 5
# speedup vs baseline: 1.6506x; 1.0883x over previous
"""GRU cell (B=4096, H=2048) on 8 TRN2 NeuronCores.

Sharding: data-parallel over the batch dim — each core computes 512 rows.
Weights are replicated; no collectives.

Per-core compute runs in "transposed" space (hidden on partitions, batch on
the free dim). Precision strategy (gate rel-err < 2e-2; 1.57e-2 in numpy
simulation of this exact scheme):
  - r/z gates and the n-gate's hh half (gh2): fp8-e4m3 DoubleRow matmuls
    (2 contraction rows per PE cell, 2x MAC rate). Acts scaled by SX=32,
    weights by SW=8192 to sit in e4m3's normal range. Sigmoid squashes the
    r/z error; gh2's error is damped by the multiply with r in (0,1).
  - n-gate ih half (gi2): bf16 (its error hits tanh 1:1, fp8 would blow
    the budget).
This cuts weight DMA ~3x and takes the matmul stream to the fp8 roofline
for 5 of 6 K-sweeps.

Schedule: phase S runs all 512 r/z DoubleRow matmuls back-to-back (the
bf16<->fp8 mode switch costs ~0.8us per junction, so junctions are
minimized); sigmoids land in SBUF as bf16. Phase M processes quads of 4
hidden blocks: 4x16 bf16 gi matmuls, then 4x8 DoubleRow gh2 matmuls (one
junction per quad), then the elementwise tails overlap the next quad's
matmuls. PSUM: two pools of 4 banks each (S: r/z, M: gi/gh).
"""

from contextlib import ExitStack

import ml_dtypes
import numpy as np

import concourse.bass as bass
import concourse.tile as tile
from concourse import bacc, mybir
from concourse.bass_utils import run_bass_kernel_spmd

H = 2048
B = 4096
N_CORES = 8
BL = B // N_CORES  # 512 batch rows per core
P = 128
NKB = H // P  # 16 contraction chunks of 128
ND = NKB // 2  # 8 DoubleRow chunks of 256
NNB = H // P  # 16 hidden (output) blocks
QUAD = 4
F32 = mybir.dt.float32
BF16 = mybir.dt.bfloat16
F8 = mybir.dt.float8e4

SX = 32.0  # activation quant scale
SW = 8192.0  # weight quant scale
SINV = 1.0 / (SX * SW)
F8MAX = 240.0  # TRN FP8_EXP4 max normal

# fp8 weight matrix order: 0: W_ih[0] (r)   1: W_hh[0] (r)
#                          2: W_ih[1] (z)   3: W_hh[1] (z)
#                          4: W_hh[2] (n, hh half)
# bf16 weights: W_ih[2] (n, ih half)


def _build_program() -> bacc.Bacc:
    nc = bacc.Bacc(
        "TRN2", target_bir_lowering=False, debug=False, num_devices=N_CORES
    )

    xq8 = nc.dram_tensor("xq8", [P, NKB, BL], F8, kind="ExternalInput").ap()
    hq8 = nc.dram_tensor("hq8", [P, NKB, BL], F8, kind="ExternalInput").ap()
    xb = nc.dram_tensor("xb", [P, NKB, BL], BF16, kind="ExternalInput").ap()
    hb = nc.dram_tensor("hb", [P, NKB, BL], BF16, kind="ExternalInput").ap()
    w8 = nc.dram_tensor("w8", [5, NNB, P, NKB, P], F8, kind="ExternalInput").ap()
    wb = nc.dram_tensor("wb", [NNB, P, NKB, P], BF16, kind="ExternalInput").ap()
    b = nc.dram_tensor("b", [P, 4 * NNB], F32, kind="ExternalInput").ap()
    out = nc.dram_tensor("out", [H, BL], F32, kind="ExternalOutput").ap()

    with tile.TileContext(nc) as tc, ExitStack() as ctx:
        const = ctx.enter_context(tc.tile_pool(name="const", bufs=1))
        acts = ctx.enter_context(tc.tile_pool(name="acts", bufs=1))
        w8pool = ctx.enter_context(tc.tile_pool(name="w8pool", bufs=10))
        wbpool = ctx.enter_context(tc.tile_pool(name="wbpool", bufs=6))
        rzsave = ctx.enter_context(tc.tile_pool(name="rzsave", bufs=NNB))
        gates = ctx.enter_context(tc.tile_pool(name="gates", bufs=2))
        opool = ctx.enter_context(tc.tile_pool(name="opool", bufs=3))
        # Two PSUM pools of 4 banks: ps_a holds r (phase S) / gi (phase M),
        # ps_b holds z / gh.
        ps_a = ctx.enter_context(tc.tile_pool(name="ps_a", bufs=4, space="PSUM"))
        ps_b = ctx.enter_context(tc.tile_pool(name="ps_b", bufs=4, space="PSUM"))

        btile = const.tile([P, 4 * NNB], F32)
        nc.scalar.dma_start(btile[:], b[:])

        xq8_sb = acts.tile([P, NKB, BL], F8)
        hq8_sb = acts.tile([P, NKB, BL], F8)
        xb_sb = acts.tile([P, NKB, BL], BF16)
        hb_sb = acts.tile([P, NKB, BL], BF16)

        # Phase-S weight slabs, need-ordered on the sync ring. nb0's r/z-ih
        # slabs chase xq8 so DoubleRow matmuls start after ~1.5 MiB.
        s8 = {}
        nc.sync.dma_start(xq8_sb[:], xq8[:])
        for m in (0, 2):
            s = w8pool.tile([P, NKB, P], F8, tag="w8", name=f"w8_{m}_0")
            nc.sync.dma_start(s[:], w8[m, 0])
            s8[(m, 0)] = s
        nc.sync.dma_start(hq8_sb[:], hq8[:])
        for m in (1, 3):
            s = w8pool.tile([P, NKB, P], F8, tag="w8", name=f"w8_{m}_0")
            nc.sync.dma_start(s[:], w8[m, 0])
            s8[(m, 0)] = s
        # bf16 acts for phase M stream on the scalar ring in parallel.
        for c in range(2):
            nc.scalar.dma_start(
                xb_sb[:, c * 8 : (c + 1) * 8, :], xb[:, c * 8 : (c + 1) * 8, :]
            )
        for c in range(2):
            nc.scalar.dma_start(
                hb_sb[:, c * 8 : (c + 1) * 8, :], hb[:, c * 8 : (c + 1) * 8, :]
            )

        # PE warm-up while the first DMAs land (HAM clock-gate release).
        warm = const.tile([P, BL], BF16)
        nc.gpsimd.memset(warm[:], 0.0)
        p_warm = ps_b.tile([P, BL], F32, tag="p_b", name="p_warm")
        for _ in range(12):
            nc.tensor.matmul(
                p_warm[:], lhsT=warm[:, :P], rhs=warm[:], start=True, stop=True
            )

        def mm_dr(psum, slab, act_sb, start, stop):
            """fp8 DoubleRow K-sweep half: 8 matmuls, K=256 each."""
            for k2 in range(ND):
                nc.tensor.matmul(
                    psum[:],
                    lhsT=slab[:, 2 * k2 : 2 * k2 + 2, :],
                    rhs=act_sb[:, 2 * k2 : 2 * k2 + 2, :],
                    start=(start and k2 == 0),
                    stop=(stop and k2 == ND - 1),
                    perf_mode=mybir.MatmulPerfMode.DoubleRow,
                )

        def mm_bf(psum, slab, act_sb, start, stop):
            """bf16 K-sweep half: 16 matmuls, K=128 each."""
            for k in range(NKB):
                nc.tensor.matmul(
                    psum[:],
                    lhsT=slab[:, k : k + 1, :],
                    rhs=act_sb[:, k : k + 1, :],
                    start=(start and k == 0),
                    stop=(stop and k == NKB - 1),
                )

        def bias_ap(g, nb):
            return btile[:, g * NNB + nb : g * NNB + nb + 1]

        # ---- Phase S: all r/z DoubleRow matmuls, sigmoids saved as bf16.
        rs = [None] * NNB
        zs = [None] * NNB
        for nb in range(NNB):
            if nb > 0:
                for m in range(4):
                    s = w8pool.tile([P, NKB, P], F8, tag="w8",
                                    name=f"w8_{m}_{nb}")
                    nc.sync.dma_start(s[:], w8[m, nb])
                    s8[(m, nb)] = s
            p_r = ps_a.tile([P, BL], F32, tag="p_a", name=f"p_r{nb}")
            p_z = ps_b.tile([P, BL], F32, tag="p_b", name=f"p_z{nb}")
            mm_dr(p_r, s8[(0, nb)], xq8_sb, True, False)
            mm_dr(p_z, s8[(2, nb)], xq8_sb, True, False)
            mm_dr(p_r, s8[(1, nb)], hq8_sb, False, True)
            mm_dr(p_z, s8[(3, nb)], hq8_sb, False, True)
            s8.pop((0, nb)); s8.pop((1, nb)); s8.pop((2, nb)); s8.pop((3, nb))
            rs[nb] = rzsave.tile([P, BL], BF16, tag="rs", name=f"rs{nb}")
            zs[nb] = rzsave.tile([P, BL], BF16, tag="zs", name=f"zs{nb}")
            nc.scalar.activation(
                rs[nb][:], p_r[:], mybir.ActivationFunctionType.Sigmoid,
                bias=bias_ap(0, nb), scale=SINV,
            )
            nc.scalar.activation(
                zs[nb][:], p_z[:], mybir.ActivationFunctionType.Sigmoid,
                bias=bias_ap(1, nb), scale=SINV,
            )

        # ---- Phase M: quads of (4x gi bf16, 4x gh2 DoubleRow, 4x tail).
        for q0 in range(0, NNB, QUAD):
            quad = range(q0, q0 + QUAD)
            slb = {}
            s8h = {}
            for nb in quad:
                s = wbpool.tile([P, NKB, P], BF16, tag="wb", name=f"wb_{nb}")
                nc.sync.dma_start(s[:], wb[nb])
                slb[nb] = s
            for nb in quad:
                s = w8pool.tile([P, NKB, P], F8, tag="w8", name=f"w8_4_{nb}")
                nc.sync.dma_start(s[:], w8[4, nb])
                s8h[nb] = s
            p_gi = {}
            p_gh = {}
            for nb in quad:
                p_gi[nb] = ps_a.tile([P, BL], F32, tag="p_a", name=f"p_gi{nb}")
                mm_bf(p_gi[nb], slb[nb], xb_sb, True, True)
            for nb in quad:
                p_gh[nb] = ps_b.tile([P, BL], F32, tag="p_b", name=f"p_gh{nb}")
                mm_dr(p_gh[nb], s8h[nb], hq8_sb, True, True)
            for nb in quad:
                last = nb == NNB - 1
                halves = 4 if last else 1
                CW = BL // halves
                # u = gh2 + b_hh2 (descaled from the fp8 psum on ScalarE)
                u_sb = gates.tile([P, BL], F32, tag="u")
                n_sb = gates.tile([P, BL], F32, tag="n")
                t_sb = gates.tile([P, BL], F32, tag="t")
                x_sb = gates.tile([P, BL], F32, tag="x")
                d_sb = gates.tile([P, BL], F32, tag="d")
                e_sb = gates.tile([P, BL], F32, tag="e")
                o_sb = opool.tile([P, BL], F32, tag="o")
                for h in range(halves):
                    hs = slice(h * CW, (h + 1) * CW)
                    nc.scalar.activation(
                        u_sb[:, hs], p_gh[nb][:, hs],
                        mybir.ActivationFunctionType.Identity,
                        bias=bias_ap(3, nb), scale=SINV,
                    )
                    # t = u * r;  x = t + gi2;  n = tanh(x + b_ih2)
                    nc.vector.tensor_mul(t_sb[:, hs], u_sb[:, hs], rs[nb][:, hs])
                    nc.vector.tensor_add(x_sb[:, hs], t_sb[:, hs], p_gi[nb][:, hs])
                    nc.scalar.activation(
                        n_sb[:, hs], x_sb[:, hs],
                        mybir.ActivationFunctionType.Tanh,
                        bias=bias_ap(2, nb),
                    )
                    # out = n + z * (hx - n)
                    nc.vector.tensor_sub(
                        d_sb[:, hs], hb_sb[:, nb, hs], n_sb[:, hs]
                    )
                    nc.vector.tensor_mul(e_sb[:, hs], zs[nb][:, hs], d_sb[:, hs])
                    nc.vector.tensor_add(o_sb[:, hs], n_sb[:, hs], e_sb[:, hs])
                    if last:
                        nc.sync.dma_start(
                            out[nb * P : (nb + 1) * P, hs], o_sb[:, hs]
                        )
                if not last:
                    nc.gpsimd.dma_start(out[nb * P : (nb + 1) * P, :], o_sb[:])

    nc.compile()
    return nc


def _pack_inputs(input, hx, weight_ih, weight_hh, bias_ih, bias_hh):
    """Host-side shard + quantize + layout packing. Per-core input maps."""
    input = np.ascontiguousarray(np.asarray(input, dtype=np.float32))
    hx = np.ascontiguousarray(np.asarray(hx, dtype=np.float32))
    weight_ih = np.asarray(weight_ih, dtype=np.float32)
    weight_hh = np.asarray(weight_hh, dtype=np.float32)
    bias_ih = np.asarray(bias_ih, dtype=np.float32)
    bias_hh = np.asarray(bias_hh, dtype=np.float32)

    E4 = ml_dtypes.float8_e4m3

    def wpack(wm):
        # [kp, k, n] per nb: [nb, kp, k, n] = W[k*128+kp, nb*128+n]
        return wm.reshape(NKB, P, NNB, P).transpose(2, 1, 0, 3)

    ws8 = [weight_ih[0], weight_hh[0], weight_ih[1], weight_hh[1], weight_hh[2]]
    w8pack = np.ascontiguousarray(
        np.stack([wpack(np.clip(wm * SW, -F8MAX, F8MAX)) for wm in ws8])
        .astype(E4)
    )
    wbpack = np.ascontiguousarray(wpack(weight_ih[2]).astype(ml_dtypes.bfloat16))

    # bpack[p, g*16+nb] = bias_g[nb*128+p]; g: r_sum, z_sum, ih2, hh2
    bias_all = np.stack(
        [bias_ih[0] + bias_hh[0], bias_ih[1] + bias_hh[1], bias_ih[2], bias_hh[2]]
    )  # [4, H]
    bpack = np.ascontiguousarray(
        bias_all.reshape(4, NNB, P).transpose(2, 0, 1).reshape(P, 4 * NNB)
    )

    def t_pack(a, dt, scale=None):
        # [BL, H] -> [P, NKB, BL] with [p, k, m] = a[m, k*128+p]
        t = a.T.reshape(NKB, P, BL).transpose(1, 0, 2)
        if scale is not None:
            t = np.clip(t * scale, -F8MAX, F8MAX)
        return np.ascontiguousarray(t.astype(dt))

    in_maps = []
    for c in range(N_CORES):
        sl = slice(c * BL, (c + 1) * BL)
        in_maps.append(
            {
                "xq8": t_pack(input[sl], E4, SX),
                "hq8": t_pack(hx[sl], E4, SX),
                "xb": t_pack(input[sl], ml_dtypes.bfloat16),
                "hb": t_pack(hx[sl], ml_dtypes.bfloat16),
                "w8": w8pack,
                "wb": wbpack,
                "b": bpack,
            }
        )
    return in_maps


_PROGRAM_CACHE = []


def kernel(input, hx, weight_ih, weight_hh, bias_ih, bias_hh, _trace=False):
    if not _PROGRAM_CACHE:
        _PROGRAM_CACHE.append(_build_program())
    nc = _PROGRAM_CACHE[0]
    in_maps = _pack_inputs(input, hx, weight_ih, weight_hh, bias_ih, bias_hh)
    res = run_bass_kernel_spmd(nc, in_maps, list(range(N_CORES)), trace=_trace)
    out = np.empty((B, H), dtype=np.float32)
    for c in range(N_CORES):
        out[c * BL : (c + 1) * BL] = res.results[c]["out"].T
    if _trace:
        kernel.last_exec_time_ns = res.exec_time_ns
    return out


# revision 10
# speedup vs baseline: 1.7729x; 1.0741x over previous
"""GRU cell (B=4096, H=2048) on 8 TRN2 NeuronCores.

Sharding: data-parallel over the batch dim — each core computes 512 rows.
Weights are replicated; no collectives.

Per-core compute runs in "transposed" space (hidden on partitions, batch on
the free dim). Precision strategy (gate rel-err < 2e-2; 1.57e-2 in numpy
simulation of this exact scheme):
  - r/z gates and the n-gate's hh half (gh2): fp8-e4m3 DoubleRow matmuls
    (2 contraction rows per PE cell, 2x MAC rate). Acts scaled by SX=32,
    weights by SW=8192 to sit in e4m3's normal range. Sigmoid squashes the
    r/z error; gh2's error is damped by the multiply with r in (0,1).
  - n-gate ih half (gi2): bf16 (its error hits tanh 1:1, fp8 would blow
    the budget).
This cuts weight DMA ~3x and takes the matmul stream to the fp8 roofline
for 5 of 6 K-sweeps.

Schedule: phase S runs all 512 r/z DoubleRow matmuls back-to-back (the
bf16<->fp8 mode switch costs ~0.8us per junction, so junctions are
minimized); sigmoids land in SBUF as bf16. Phase M processes quads of 4
hidden blocks: 4x16 bf16 gi matmuls, then 4x8 DoubleRow gh2 matmuls (one
junction per quad), then the elementwise tails overlap the next quad's
matmuls. PSUM: two pools of 4 banks each (S: r/z, M: gi/gh).
"""

from contextlib import ExitStack

import ml_dtypes
import numpy as np

import concourse.bass as bass
import concourse.tile as tile
from concourse import bacc, mybir
from concourse.bass_utils import run_bass_kernel_spmd

H = 2048
B = 4096
N_CORES = 8
BL = B // N_CORES  # 512 batch rows per core
P = 128
NKB = H // P  # 16 contraction chunks of 128
ND = NKB // 2  # 8 DoubleRow chunks of 256
NNB = H // P  # 16 hidden (output) blocks
QUAD = 4
F32 = mybir.dt.float32
BF16 = mybir.dt.bfloat16
F8 = mybir.dt.float8e4

SX = 32.0  # activation quant scale
SW = 8192.0  # weight quant scale
SINV = 1.0 / (SX * SW)
F8MAX = 240.0  # TRN FP8_EXP4 max normal

# fp8 weight matrix order: 0: W_ih[0] (r)   1: W_hh[0] (r)
#                          2: W_ih[1] (z)   3: W_hh[1] (z)
#                          4: W_hh[2] (n, hh half)
# bf16 weights: W_ih[2] (n, ih half)


def _build_program() -> bacc.Bacc:
    nc = bacc.Bacc(
        "TRN2", target_bir_lowering=False, debug=False, num_devices=N_CORES
    )

    xq8 = nc.dram_tensor("xq8", [P, NKB, BL], F8, kind="ExternalInput").ap()
    hq8 = nc.dram_tensor("hq8", [P, NKB, BL], F8, kind="ExternalInput").ap()
    xb = nc.dram_tensor("xb", [P, NKB, BL], BF16, kind="ExternalInput").ap()
    hb = nc.dram_tensor("hb", [P, NKB, BL], BF16, kind="ExternalInput").ap()
    w8 = nc.dram_tensor("w8", [5, NNB, P, NKB, P], F8, kind="ExternalInput").ap()
    wb = nc.dram_tensor("wb", [NNB, P, NKB, P], BF16, kind="ExternalInput").ap()
    b = nc.dram_tensor("b", [P, 4 * NNB], F32, kind="ExternalInput").ap()
    out = nc.dram_tensor("out", [H, BL], BF16, kind="ExternalOutput").ap()

    with tile.TileContext(nc) as tc, ExitStack() as ctx:
        const = ctx.enter_context(tc.tile_pool(name="const", bufs=1))
        acts = ctx.enter_context(tc.tile_pool(name="acts", bufs=1))
        w8pool = ctx.enter_context(tc.tile_pool(name="w8pool", bufs=10))
        wbpool = ctx.enter_context(tc.tile_pool(name="wbpool", bufs=6))
        rzsave = ctx.enter_context(tc.tile_pool(name="rzsave", bufs=NNB))
        gates = ctx.enter_context(tc.tile_pool(name="gates", bufs=2))
        opool = ctx.enter_context(tc.tile_pool(name="opool", bufs=3))
        # Two PSUM pools of 4 banks: ps_a holds r (phase S) / gi (phase M),
        # ps_b holds z / gh.
        ps_a = ctx.enter_context(tc.tile_pool(name="ps_a", bufs=4, space="PSUM"))
        ps_b = ctx.enter_context(tc.tile_pool(name="ps_b", bufs=4, space="PSUM"))

        btile = const.tile([P, 4 * NNB], F32)
        nc.scalar.dma_start(btile[:], b[:])

        xq8_sb = acts.tile([P, NKB, BL], F8)
        hq8_sb = acts.tile([P, NKB, BL], F8)
        xb_sb = acts.tile([P, NKB, BL], BF16)
        hb_sb = acts.tile([P, NKB, BL], BF16)

        # Phase-S weight slabs, need-ordered on the sync ring. nb0's r/z-ih
        # slabs chase xq8 so DoubleRow matmuls start after ~1.5 MiB.
        s8 = {}
        nc.sync.dma_start(xq8_sb[:], xq8[:])
        for m in (0, 2):
            s = w8pool.tile([P, NKB, P], F8, tag="w8", name=f"w8_{m}_0")
            nc.sync.dma_start(s[:], w8[m, 0])
            s8[(m, 0)] = s
        nc.sync.dma_start(hq8_sb[:], hq8[:])
        for m in (1, 3):
            s = w8pool.tile([P, NKB, P], F8, tag="w8", name=f"w8_{m}_0")
            nc.sync.dma_start(s[:], w8[m, 0])
            s8[(m, 0)] = s
        # bf16 acts for phase M are interleaved into the S-phase sync
        # stream (nb 1..4) so they don't contend with the startup fp8 bytes.

        # PE warm-up while the first DMAs land (HAM clock-gate release).
        warm = const.tile([P, BL], BF16)
        nc.gpsimd.memset(warm[:], 0.0)
        p_warm = ps_b.tile([P, BL], F32, tag="p_b", name="p_warm")
        for _ in range(12):
            nc.tensor.matmul(
                p_warm[:], lhsT=warm[:, :P], rhs=warm[:], start=True, stop=True
            )

        def mm_dr(psum, slab, act_sb, start, stop):
            """fp8 DoubleRow K-sweep half: 8 matmuls, K=256 each."""
            for k2 in range(ND):
                nc.tensor.matmul(
                    psum[:],
                    lhsT=slab[:, 2 * k2 : 2 * k2 + 2, :],
                    rhs=act_sb[:, 2 * k2 : 2 * k2 + 2, :],
                    start=(start and k2 == 0),
                    stop=(stop and k2 == ND - 1),
                    perf_mode=mybir.MatmulPerfMode.DoubleRow,
                )

        def mm_bf(psum, slab, act_sb, start, stop):
            """bf16 K-sweep half: 16 matmuls, K=128 each."""
            for k in range(NKB):
                nc.tensor.matmul(
                    psum[:],
                    lhsT=slab[:, k : k + 1, :],
                    rhs=act_sb[:, k : k + 1, :],
                    start=(start and k == 0),
                    stop=(stop and k == NKB - 1),
                )

        def bias_ap(g, nb):
            return btile[:, g * NNB + nb : g * NNB + nb + 1]

        # ---- Phase S: all r/z DoubleRow matmuls, sigmoids saved as bf16.
        rs = [None] * NNB
        zs = [None] * NNB
        mact = [xb_sb, xb_sb, hb_sb, hb_sb]
        for nb in range(NNB):
            if nb > 0:
                for m in range(4):
                    s = w8pool.tile([P, NKB, P], F8, tag="w8",
                                    name=f"w8_{m}_{nb}")
                    nc.sync.dma_start(s[:], w8[m, nb])
                    s8[(m, nb)] = s
            if 1 <= nb <= 4:
                t = mact[nb - 1]
                src = xb if nb <= 2 else hb
                c = (nb - 1) % 2
                nc.sync.dma_start(
                    t[:, c * 8 : (c + 1) * 8, :], src[:, c * 8 : (c + 1) * 8, :]
                )
            p_r = ps_a.tile([P, BL], F32, tag="p_a", name=f"p_r{nb}")
            p_z = ps_b.tile([P, BL], F32, tag="p_b", name=f"p_z{nb}")
            mm_dr(p_r, s8[(0, nb)], xq8_sb, True, False)
            mm_dr(p_z, s8[(2, nb)], xq8_sb, True, False)
            mm_dr(p_r, s8[(1, nb)], hq8_sb, False, True)
            mm_dr(p_z, s8[(3, nb)], hq8_sb, False, True)
            s8.pop((0, nb)); s8.pop((1, nb)); s8.pop((2, nb)); s8.pop((3, nb))
            rs[nb] = rzsave.tile([P, BL], BF16, tag="rs", name=f"rs{nb}")
            zs[nb] = rzsave.tile([P, BL], BF16, tag="zs", name=f"zs{nb}")
            nc.scalar.activation(
                rs[nb][:], p_r[:], mybir.ActivationFunctionType.Sigmoid,
                bias=bias_ap(0, nb), scale=SINV,
            )
            nc.scalar.activation(
                zs[nb][:], p_z[:], mybir.ActivationFunctionType.Sigmoid,
                bias=bias_ap(1, nb), scale=SINV,
            )

        # ---- Phase M: quads of (4x gi bf16, 4x gh2 DoubleRow, 4x tail).
        for q0 in range(0, NNB, QUAD):
            quad = range(q0, q0 + QUAD)
            slb = {}
            s8h = {}
            for nb in quad:
                s = wbpool.tile([P, NKB, P], BF16, tag="wb", name=f"wb_{nb}")
                nc.sync.dma_start(s[:], wb[nb])
                slb[nb] = s
            for nb in quad:
                s = w8pool.tile([P, NKB, P], F8, tag="w8", name=f"w8_4_{nb}")
                nc.sync.dma_start(s[:], w8[4, nb])
                s8h[nb] = s
            p_gi = {}
            for nb in quad:
                p_gi[nb] = ps_a.tile([P, BL], F32, tag="p_a", name=f"p_gi{nb}")
                mm_bf(p_gi[nb], slb[nb], xb_sb, True, True)
            # gh2 DoubleRow + tail interleaved per block: tail(nb) overlaps
            # gh(nb+1)'s matmuls, so only the final block's tail is exposed.
            for nb in quad:
                p_gh = ps_b.tile([P, BL], F32, tag="p_b", name=f"p_gh{nb}")
                mm_dr(p_gh, s8h[nb], hq8_sb, True, True)
                last = nb == NNB - 1
                halves = 4 if last else 1
                CW = BL // halves
                # Tail in bf16 (2x DVE rate): u = gh2 + b_hh2 (descaled),
                # t = u*r, x = t + gi2 (f32: psum operand), n = tanh(x+b),
                # out = n + z*(hx - n).
                u_sb = gates.tile([P, BL], BF16, tag="u")
                t_sb = gates.tile([P, BL], BF16, tag="t")
                x_sb = gates.tile([P, BL], F32, tag="x")
                n_sb = gates.tile([P, BL], BF16, tag="n")
                d_sb = gates.tile([P, BL], BF16, tag="d")
                e_sb = gates.tile([P, BL], BF16, tag="e")
                o_sb = opool.tile([P, BL], BF16, tag="o")
                for h in range(halves):
                    hs = slice(h * CW, (h + 1) * CW)
                    nc.scalar.activation(
                        u_sb[:, hs], p_gh[:, hs],
                        mybir.ActivationFunctionType.Identity,
                        bias=bias_ap(3, nb), scale=SINV,
                    )
                    nc.vector.tensor_mul(t_sb[:, hs], u_sb[:, hs], rs[nb][:, hs])
                    nc.vector.tensor_add(x_sb[:, hs], t_sb[:, hs], p_gi[nb][:, hs])
                    nc.scalar.activation(
                        n_sb[:, hs], x_sb[:, hs],
                        mybir.ActivationFunctionType.Tanh,
                        bias=bias_ap(2, nb),
                    )
                    nc.vector.tensor_sub(
                        d_sb[:, hs], hb_sb[:, nb, hs], n_sb[:, hs]
                    )
                    nc.vector.tensor_mul(e_sb[:, hs], zs[nb][:, hs], d_sb[:, hs])
                    nc.vector.tensor_add(o_sb[:, hs], n_sb[:, hs], e_sb[:, hs])
                    if last:
                        nc.sync.dma_start(
                            out[nb * P : (nb + 1) * P, hs], o_sb[:, hs]
                        )
                if not last:
                    nc.gpsimd.dma_start(out[nb * P : (nb + 1) * P, :], o_sb[:])

    nc.compile()
    return nc


def _pack_inputs(input, hx, weight_ih, weight_hh, bias_ih, bias_hh):
    """Host-side shard + quantize + layout packing. Per-core input maps."""
    input = np.ascontiguousarray(np.asarray(input, dtype=np.float32))
    hx = np.ascontiguousarray(np.asarray(hx, dtype=np.float32))
    weight_ih = np.asarray(weight_ih, dtype=np.float32)
    weight_hh = np.asarray(weight_hh, dtype=np.float32)
    bias_ih = np.asarray(bias_ih, dtype=np.float32)
    bias_hh = np.asarray(bias_hh, dtype=np.float32)

    E4 = ml_dtypes.float8_e4m3

    def wpack(wm):
        # [kp, k, n] per nb: [nb, kp, k, n] = W[k*128+kp, nb*128+n]
        return wm.reshape(NKB, P, NNB, P).transpose(2, 1, 0, 3)

    ws8 = [weight_ih[0], weight_hh[0], weight_ih[1], weight_hh[1], weight_hh[2]]
    w8pack = np.ascontiguousarray(
        np.stack([wpack(np.clip(wm * SW, -F8MAX, F8MAX)) for wm in ws8])
        .astype(E4)
    )
    wbpack = np.ascontiguousarray(wpack(weight_ih[2]).astype(ml_dtypes.bfloat16))

    # bpack[p, g*16+nb] = bias_g[nb*128+p]; g: r_sum, z_sum, ih2, hh2
    bias_all = np.stack(
        [bias_ih[0] + bias_hh[0], bias_ih[1] + bias_hh[1], bias_ih[2], bias_hh[2]]
    )  # [4, H]
    bpack = np.ascontiguousarray(
        bias_all.reshape(4, NNB, P).transpose(2, 0, 1).reshape(P, 4 * NNB)
    )

    def t_pack(a, dt, scale=None):
        # [BL, H] -> [P, NKB, BL] with [p, k, m] = a[m, k*128+p]
        t = a.T.reshape(NKB, P, BL).transpose(1, 0, 2)
        if scale is not None:
            t = np.clip(t * scale, -F8MAX, F8MAX)
        return np.ascontiguousarray(t.astype(dt))

    in_maps = []
    for c in range(N_CORES):
        sl = slice(c * BL, (c + 1) * BL)
        in_maps.append(
            {
                "xq8": t_pack(input[sl], E4, SX),
                "hq8": t_pack(hx[sl], E4, SX),
                "xb": t_pack(input[sl], ml_dtypes.bfloat16),
                "hb": t_pack(hx[sl], ml_dtypes.bfloat16),
                "w8": w8pack,
                "wb": wbpack,
                "b": bpack,
            }
        )
    return in_maps


_PROGRAM_CACHE = []


def kernel(input, hx, weight_ih, weight_hh, bias_ih, bias_hh, _trace=False):
    if not _PROGRAM_CACHE:
        _PROGRAM_CACHE.append(_build_program())
    nc = _PROGRAM_CACHE[0]
    in_maps = _pack_inputs(input, hx, weight_ih, weight_hh, bias_ih, bias_hh)
    res = run_bass_kernel_spmd(nc, in_maps, list(range(N_CORES)), trace=_trace)
    out = np.empty((B, H), dtype=np.float32)
    for c in range(N_CORES):
        out[c * BL : (c + 1) * BL] = res.results[c]["out"].T.astype(np.float32)
    if _trace:
        kernel.last_exec_time_ns = res.exec_time_ns
    return out
